# revision 19
# baseline (speedup 1.0000x reference)
"""Trainium2 Bass kernel for nn_CrossAttention (B=4, Q=1024, T=4096, D=1024, H=16).

Sharding: core = b*2 + g  (b in 0..3 batches, g in 0..1 head-groups of 8 heads).
Each core computes, for its (batch, head-group):
  qT = (Wq_g @ x_q.T)          [512, Q]   (feature-major; head pairs stacked)
  kT = (Wk_g @ x_kv.T)         [512, T]
  v  = (x_kv @ Wv_g.T)         [T, 512]
  sT = k_h @ q_h.T             [T, Q] per head  (scores transposed)
  p  = exp(sT / 8)             (softmax w/o max-subtraction; scores ~N(0,1))
  outT_h = v_h.T @ p           (PV accumulated in PSUM)
  sums_h = column-sums of p    (DVE+GpSimd fp16 running accumulator + one PE
                                ones-matmul reduce, which also broadcasts)
  attnT_h = outT_h * (1/sums_h)
  yT += Wo[:, gblock].T.T @ attnT  (pairs 0-2 partial and pair-3 final stored
                                    separately in bf16; host sums)
Host sums the partials per batch and transposes.

The schedule is exp-paced: ScalarE's exp throughput (~1.1us per t-tile of two
heads) is the hard floor, so projection work is spread so that no phase's PE
backlog exceeds its exp budget (~35.6us).  v-proj is sliced by head-pair
columns (pair 0 inside phase (0,0), pair 1 in phases 2-3, pairs 2-3 in
phases 4-5); k-proj chunks carry per-chunk deadlines spanning two phases per
pair; o-projection partials are DMA'd out (bf16) as soon as their inputs
exist and summed on the host.  A chain of dummy matmuls at the top warms the
PE HAM clock gate during the initial DMA wait.
"""

import os
import sys

import numpy as np

# The device can enter a persistent ~20% clock-throttle state after long
# sessions; a core reset at runtime init restores nominal speed.
os.environ.setdefault("NEURON_RT_RESET_CORES", "1")

for _p in ("/opt/trn_rl_repo",):
    if _p not in sys.path:
        sys.path.insert(0, _p)

import ml_dtypes

import concourse.bass as bass
import concourse.tile as tile
from concourse import bacc, mybir
from concourse.bass_utils import run_bass_kernel_spmd

BF16 = mybir.dt.bfloat16
F16 = mybir.dt.float16
F32 = mybir.dt.float32
NPBF16 = np.dtype(ml_dtypes.bfloat16)

D = 1024          # model dim
Q = 1024          # query length
T = 4096          # kv length
B = 4             # batch
H = 16            # heads
DH = 64           # head dim
NCORES = 8
G = 2             # head groups (cores per batch)
F = D // G        # features per core = 512
P = 128
ND = D // P       # 8 d-tiles (contraction tiles for projections)
NM = F // P       # 4 feature tiles (head pairs)
NQC = Q // 512    # 2 query chunks
NTC = T // 512    # 8 kv chunks
NTT = T // P      # 32 kv tiles
SCALE = DH ** -0.5


def _emit_kernel(nc, tc, xqT, xkT, wq0, wq1, wk0, wk1, wvT, woT, yT, yT2):
    from contextlib import ExitStack

    ctx = ExitStack()
    with ctx:
        wp = ctx.enter_context(tc.tile_pool(name="wp", bufs=1))
        xp = ctx.enter_context(tc.tile_pool(name="xp", bufs=4))
        xqp = ctx.enter_context(tc.tile_pool(name="xqp", bufs=2))
        st = ctx.enter_context(tc.tile_pool(name="st", bufs=1))
        exp_pool = ctx.enter_context(tc.tile_pool(name="exp", bufs=4))
        accp = ctx.enter_context(tc.tile_pool(name="accp", bufs=2))
        small = ctx.enter_context(tc.tile_pool(name="small", bufs=2))
        yop = ctx.enter_context(tc.tile_pool(name="yop", bufs=4))
        psp = ctx.enter_context(tc.tile_pool(name="psp", bufs=1, space="PSUM"))

        # ---- resident weights / activations ----
        wq0_sb = wp.tile([P, ND, P], BF16, name="wq0_sb", tag="wq0")
        wq1_sb = wp.tile([P, ND, F - P], BF16, name="wq1_sb", tag="wq1")
        wk0_sb = wp.tile([P, ND, P], BF16, name="wk0_sb", tag="wk0")
        wk1_sb = wp.tile([P, ND, F - P], BF16, name="wk1_sb", tag="wk1")
        wv_sb = wp.tile([P, ND, F], BF16, name="wv_sb", tag="wv")
        wo_sb = wp.tile([P, NM, D], BF16, name="wo_sb", tag="wo")
        qT_sb = st.tile([P, NM, Q], BF16, name="qT_sb", tag="qT")
        kT_sb = st.tile([P, NM, T], BF16, name="kT_sb", tag="kT")
        v_sb = st.tile([P, NTT, F], BF16, name="v_sb", tag="v")
        at_sb = st.tile([P, NM, Q], BF16, name="at_sb", tag="at")
        ones64 = st.tile([P, DH], F16, name="ones64", tag="ones")

        def wsel_q(m):
            return (wq0_sb, 0) if m == 0 else (wq1_sb, (m - 1) * P)

        def wsel_k(p):
            return (wk0_sb, 0) if p == 0 else (wk1_sb, (p - 1) * P)

        def xk_dma(tc_i, eng=None):
            """Load one 512-col chunk of xkT; returns the tile.  Chunk loads
            alternate between the Sync and GpSimd DMA queues so the xk
            stream isn't bound by a single queue's bandwidth."""
            xk2 = xp.tile([P, ND, 512], BF16, name="xk2", tag="xk2")
            if eng is None:
                eng = nc.sync if tc_i % 2 == 0 else nc.gpsimd
            eng.dma_start(out=xk2, in_=xkT[:, tc_i:tc_i + 1, :, :])
            return xk2

        # ---- projection emitters: (pre_action, [compute actions]) ----
        # xk_get: callable resolving the input chunk tile at run time (for
        # chunk sharing between k-proj and v-proj); if None, a private DMA
        # pre-action is returned.
        def kproj_chunk(p, tc_i, xk_get=None, c0=0, c1=512):
            state = {}

            def dma():
                state["xk"] = xk_dma(tc_i)
            getx = xk_get if xk_get is not None else (lambda: state["xk"])

            comp = []

            def alloc():
                state["pk"] = psp.tile([P, c1 - c0], F32, name="pk", tag="pp",
                                       bufs=2)

            comp.append(alloc)
            w_sb, woff = wsel_k(p)
            for d in range(ND):
                def mm(d=d):
                    nc.tensor.matmul(
                        state["pk"],
                        lhsT=w_sb[:, d, woff:woff + P],
                        rhs=getx()[:, d, c0:c1],
                        start=(d == 0),
                        stop=(d == ND - 1),
                    )
                comp.append(mm)

            def cp():
                nc.vector.tensor_copy(
                    out=kT_sb[:, p, tc_i * 512 + c0:tc_i * 512 + c1],
                    in_=state["pk"],
                )
            comp.append(cp)
            return (None if xk_get is not None else dma), comp

        def vproj_cols(tc_i, c0, c1, xk_get=None):
            """v columns [c0:c1) for one 512-row chunk.  For width<=128 all
            4 t-tiles accumulate in one [P, 4, w] psum bank with a single
            strided copy out; for wider slices, per-t-tile psums."""
            w = c1 - c0
            state = {}

            def dma():
                state["xk"] = xk_dma(tc_i)
            getx = xk_get if xk_get is not None else (lambda: state["xk"])

            comp = []
            if w <= P:
                def alloc():
                    state["ps"] = psp.tile([P, 4, w], F32, name="pv0",
                                           tag="pp", bufs=2)
                comp.append(alloc)
                for j in range(4):
                    for d in range(ND):
                        def mm(j=j, d=d):
                            nc.tensor.matmul(
                                state["ps"][:, j, :],
                                lhsT=getx()[:, d, j * P:(j + 1) * P],
                                rhs=wv_sb[:, d, c0:c1],
                                start=(d == 0),
                                stop=(d == ND - 1),
                            )
                        comp.append(mm)

                def cp():
                    nc.vector.tensor_copy(
                        out=v_sb[:, tc_i * 4:(tc_i + 1) * 4, c0:c1],
                        in_=state["ps"],
                    )
                comp.append(cp)
            else:
                for j in range(4):
                    def alloc(j=j):
                        state[j] = psp.tile([P, w], F32, name="pvr",
                                            tag="pp", bufs=2)
                    comp.append(alloc)
                    for d in range(ND):
                        def mm(j=j, d=d):
                            nc.tensor.matmul(
                                state[j],
                                lhsT=getx()[:, d, j * P:(j + 1) * P],
                                rhs=wv_sb[:, d, c0:c1],
                                start=(d == 0),
                                stop=(d == ND - 1),
                            )
                        comp.append(mm)

                    def cp(j=j):
                        nc.vector.tensor_copy(
                            out=v_sb[:, tc_i * 4 + j, c0:c1], in_=state[j]
                        )
                    comp.append(cp)
            return (None if xk_get is not None else dma), comp

        def qproj_dma(qc):
            xq_t = xqp.tile([P, ND, 512], BF16, name="xq_t", tag="xq")
            nc.sync.dma_start(out=xq_t, in_=xqT[:, qc:qc + 1, :, :])
            return xq_t

        def qproj_m(qc, m, xq_get):
            """Compute actions for one head-pair column block of q-proj."""
            state = {}
            comp = []

            def alloc():
                state["pq"] = psp.tile([P, 512], F32, name="pq", tag="pp",
                                       bufs=2)
            comp.append(alloc)
            w_sb, woff = wsel_q(m)
            for d in range(ND):
                def mm(d=d):
                    nc.tensor.matmul(
                        state["pq"],
                        lhsT=w_sb[:, d, woff:woff + P],
                        rhs=xq_get()[:, d, :],
                        start=(d == 0),
                        stop=(d == ND - 1),
                    )
                comp.append(mm)

            def cp():
                nc.vector.tensor_copy(
                    out=qT_sb[:, m, qc * 512:(qc + 1) * 512],
                    in_=state["pq"],
                )
            comp.append(cp)
            return comp

        # o-projection: partial = pairs 0-2 -> bf16 -> yT2 (host adds);
        # final = pair 3 -> bf16 -> yT.  No cross-dependencies, so partials
        # stream out as soon as their at_sb pairs exist.
        def oproj_partial(m8, qc):
            state = {}
            comp = []

            def alloc():
                state["py"] = psp.tile([P, 512], F32, name="pyp", tag="pp",
                                       bufs=2)
            comp.append(alloc)
            for k in range(NM - 1):
                def mm(k=k):
                    nc.tensor.matmul(
                        state["py"],
                        lhsT=wo_sb[:, k, m8 * P:(m8 + 1) * P],
                        rhs=at_sb[:, k, qc * 512:(qc + 1) * 512],
                        start=(k == 0),
                        stop=(k == NM - 2),
                    )
                comp.append(mm)

            def cp():
                yp = yop.tile([P, 512], BF16, name="yp", tag="y")
                nc.vector.tensor_copy(out=yp, in_=state["py"])
                nc.sync.dma_start(
                    out=yT2[m8 * P:(m8 + 1) * P, qc * 512:(qc + 1) * 512],
                    in_=yp,
                )
            comp.append(cp)
            return None, comp

        def oproj_final(m8, qc, ptag="pp", scalar_copy=False):
            state = {}
            comp = []

            def alloc():
                state["py"] = psp.tile([P, 512], F32, name="pyf", tag=ptag,
                                       bufs=2)
            comp.append(alloc)

            def mm():
                nc.tensor.matmul(
                    state["py"],
                    lhsT=wo_sb[:, NM - 1, m8 * P:(m8 + 1) * P],
                    rhs=at_sb[:, NM - 1, qc * 512:(qc + 1) * 512],
                    start=True,
                    stop=True,
                )
            comp.append(mm)

            def st_dma():
                y_t = yop.tile([P, 512], BF16, name="y_t", tag="y")
                if scalar_copy:
                    # coda: idle ScalarE moves PSUM->SBUF
                    nc.scalar.copy(out=y_t, in_=state["py"])
                else:
                    nc.vector.tensor_copy(out=y_t, in_=state["py"])
                nc.sync.dma_start(
                    out=yT[m8 * P:(m8 + 1) * P, qc * 512:(qc + 1) * 512],
                    in_=y_t,
                )
            comp.append(st_dma)
            return None, comp

        def run(pre, comp):
            if pre is not None:
                pre()
            for a in comp:
                a()

        def spread(pairs, nsteps, lead=4, s0=0):
            """Evenly distribute (pre, comp) groups over steps [s0, s0+nsteps);
            pre (DMA) actions are placed `lead` slots before the group's
            first compute action."""
            sched = [[] for _ in range(NTT)]
            total = sum(len(c) for _, c in pairs) or 1
            pos = 0
            for pre, comp in pairs:
                first = s0 + (pos * nsteps) // total
                if pre is not None:
                    sched[max(0, min(NTT - 1, first - lead))].append(pre)
                for a in comp:
                    sched[min(NTT - 1, s0 + (pos * nsteps) // total)
                          ].append(a)
                    pos += 1
            return sched

        def merge(*scheds):
            out = [[] for _ in range(NTT)]
            for s in scheds:
                for i, acts in enumerate(s):
                    out[min(i, NTT - 1)].extend(acts)
            return out

        # ================= prologue =================
        # HAM warmup: dummy matmuls on ones64 keep the PE busy through the
        # initial DMA wait so the first real matmuls run at 2.4 GHz.
        nc.vector.memset(ones64, 1.0)
        wps = psp.tile([P, 512], F32, name="warm", tag="pp", bufs=2)
        for i in range(96):
            nc.tensor.matmul(
                wps[0:DH, 0:DH], lhsT=ones64[:, 0:DH], rhs=ones64[:, 0:DH],
                start=True, stop=True,
            )
        # critical-path DMAs first (contiguous tensors, cheap descriptors)
        # spread the input loads over three DMA queues (each sustains only
        # ~100 GB/s): Sync gets the critical full xq; the fast-issuing
        # GpSimd queue gets the small early pieces (wq0, wk0, pair-0 wv
        # columns, xk chunk 0 in four 128-col slices); ScalarE's (slow)
        # queue gets what is needed only from phase 2 on.
        xq_hold = {0: None, 1: None}
        xq_hold[0] = qproj_dma(0)
        nc.gpsimd.dma_start(out=wq0_sb, in_=wq0[:, :, :])
        nc.gpsimd.dma_start(out=wk0_sb, in_=wk0[:, :, :])
        nc.gpsimd.dma_start(out=wv_sb[:, :, 0:P], in_=wvT[:, :, 0:P])
        xk0 = xp.tile([P, ND, 512], BF16, name="xk2", tag="xk2")
        for sl in range(4):
            nc.gpsimd.dma_start(out=xk0[:, :, sl * P:(sl + 1) * P],
                                in_=xkT[:, 0:1, :, sl * P:(sl + 1) * P])
        xk_hold = {0: xk0}
        xk_hold[1] = xk_dma(1, eng=nc.sync)
        nc.scalar.dma_start(out=wv_sb[:, :, P:F], in_=wvT[:, :, P:F])
        nc.scalar.dma_start(out=wq1_sb, in_=wq1[:, :, :])
        nc.sync.dma_start(out=wk1_sb, in_=wk1[:, :, :])
        # k-proj chunk 0 in four 128-col pieces (each starts as its xk
        # slice lands), then q-proj pair 0 (waits the full xq)
        getx0 = lambda: xk_hold[0]
        for sl in range(4):
            run(*kproj_chunk(0, 0, xk_get=getx0, c0=sl * P, c1=(sl + 1) * P))
        for a in qproj_m(0, 0, lambda: xq_hold[0]):
            a()

        # ================= filler schedules ==================
        def xk_load(c, hold):
            def act(c=c):
                hold[c] = xk_dma(c)
            return act

        def holder(hold, c):
            return lambda: hold[c]

        # phase (0,0): deadline-driven: k-proj(p0) chunk c by step 4c,
        # v-proj pair0 chunk c by step 4c+2; one shared xk load per chunk.
        p0 = [[] for _ in range(NTT)]

        def qp1dma():
            xq_hold[1] = qproj_dma(1)
        p0[0].append(qp1dma)
        vp0 = [vproj_cols(c, 0, P, xk_get=holder(xk_hold, c))
               for c in range(NTC)]
        # chunk 0 lands in steps 0-1 (PV consumes v tiles 0-3 from step 4)
        n0 = len(vp0[0][1])
        p0[0].extend(vp0[0][1][:(n0 + 1) // 2])
        p0[1].extend(vp0[0][1][(n0 + 1) // 2:])
        for c in range(1, NTC):
            if c >= 2:
                p0[max(0, 4 * c - 8)].append(xk_load(c, xk_hold))
            _, comp = kproj_chunk(0, c, xk_get=holder(xk_hold, c))
            base = 4 * (c - 1)
            n = len(comp)
            for si in range(4):
                lo, hi = (n * si) // 4, (n * (si + 1)) // 4
                p0[base + si].extend(comp[lo:hi])
            # v pair0 chunk c: steps 4c-2 .. 4c+1
            _, comp_v = vp0[c]
            nv = len(comp_v)
            for si in range(4):
                lo, hi = (nv * si) // 4, (nv * (si + 1)) // 4
                p0[min(NTT - 1, 4 * c - 2 + si)].extend(comp_v[lo:hi])
        # deferred q-proj for qc1 pair 0 (due end of this phase)
        for i, a in enumerate(qproj_m(1, 0, lambda: xq_hold[1])):
            p0[26 + (i * 6) // 10].append(a)
        sched = {(0, 0): p0}

        # phase (0,1): combined [kp1[c] + vp1[c]] groups for chunks 0-3
        # sharing one xk load each, + q-proj m1 qc0 + wo DMA.
        xk_h2 = {}

        def kv_group(p, c, vc0, vc1, hold):
            pre = xk_load(c, hold)
            _, kcomp = kproj_chunk(p, c, xk_get=holder(hold, c))
            _, vcomp = vproj_cols(c, vc0, vc1, xk_get=holder(hold, c))
            return pre, kcomp + vcomp

        sched[(0, 1)] = merge(
            spread([kv_group(1, c, P, 2 * P, xk_h2) for c in range(4)],
                   28, s0=0),
            spread([(None, qproj_m(0, 1, lambda: xq_hold[0])),
                    (lambda: nc.sync.dma_start(out=wo_sb, in_=woT[:, :, :]),
                     [])], 4, s0=28),
        )
        # phase (1,0): kp1+vp1 chunks 4-7 (kp1 deadline 4c), kp2 chunks 0-1,
        # q m1 qc1 (due before phase (1,1))
        sched[(1, 0)] = merge(
            spread([kv_group(1, c, P, 2 * P, xk_h2) for c in range(4, NTC)],
                   20, s0=0),
            spread([kproj_chunk(2, c) for c in range(0, 2)], 8, s0=20),
            spread([(None, qproj_m(1, 1, lambda: xq_hold[1]))], 4, s0=28),
        )
        # phase (1,1): kp2 chunks 2-3, vp23 chunks 0-3, q m2 qc0
        sched[(1, 1)] = merge(
            spread([kproj_chunk(2, c) for c in range(2, 4)], 8, s0=0),
            spread([vproj_cols(c, 2 * P, F) for c in range(0, 4)], 20, s0=6),
            spread([(None, qproj_m(0, 2, lambda: xq_hold[0]))], 6, s0=26),
        )
        # phase (2,0): kp2 chunks 4-7 (deadline 4c), vp23 chunks 4-7
        # (pair-2 deadline 4c as well) -- interleave by deadline order;
        # q m2 qc1 (due before phase (2,1))
        kv2 = []
        for c in range(4, NTC):
            kv2.append(kproj_chunk(2, c))
            kv2.append(vproj_cols(c, 2 * P, F))
        sched[(2, 0)] = merge(
            spread(kv2, 28, s0=0),
            spread([(None, qproj_m(1, 2, lambda: xq_hold[1]))], 4, s0=28),
        )
        # phase (2,1): kp3 chunks 0-5, q m3 qc0, o-partial qc0
        sched[(2, 1)] = merge(
            spread([kproj_chunk(3, c) for c in range(0, 6)], 18, s0=0),
            spread([(None, qproj_m(0, 3, lambda: xq_hold[0]))], 4, s0=18),
            spread([oproj_partial(m8, 0) for m8 in range(D // P)], 10,
                   s0=22),
        )
        # phase (3,0): kp3 chunks 6-7 (deadlines 24/28), q m3 qc1,
        # o-partial qc1
        sched[(3, 0)] = merge(
            spread([kproj_chunk(3, c) for c in range(6, NTC)], 8, s0=14),
            spread([oproj_partial(m8, 1) for m8 in range(D // P)], 14, s0=0),
            spread([(None, qproj_m(1, 3, lambda: xq_hold[1]))], 6, s0=24),
        )
        # phase (3,1): o-final qc0
        sched[(3, 1)] = spread(
            [oproj_final(m8, 0) for m8 in range(D // P)], NTT)

        # ================= attention (software-pipelined) ========
        # The previous phase's sums-reduce + normalize is deferred into the
        # current phase (emitted right after the first two QK groups) so
        # the next phase's first exp is never gated behind it.
        pending_finish = []
        for p in range(NM):
            for qc in range(NQC):
                qs = slice(qc * 512, (qc + 1) * 512)
                pvt = psp.tile([P, 512], F32, name="pvt", tag="pv", bufs=2)
                acc = accp.tile([P, 4, 512], F16, name="acc", tag="acc")

                def pv_sums(ex, j, t, p=p, pvt=pvt):
                    nc.tensor.matmul(
                        pvt[0:DH, :],
                        lhsT=v_sb[:, t, p * P:p * P + DH],
                        rhs=ex[:, 2 * j, :],
                        start=(t == 0),
                        stop=(t == NTT - 1),
                        tile_position=(0, 0),
                    )
                    nc.tensor.matmul(
                        pvt[DH:P, :],
                        lhsT=v_sb[:, t, p * P + DH:(p + 1) * P],
                        rhs=ex[:, 2 * j + 1, :],
                        start=(t == 0),
                        stop=(t == NTT - 1),
                        tile_position=(0, 64),
                        skip_group_check=True,
                    )

                def qk_exp(t, expair, j):
                    ts = slice(t * P, (t + 1) * P)
                    s_ps = psp.tile([P, 2, 512], F32, name="s_ps", tag="ps",
                                    bufs=2)
                    for hb in range(2):
                        base = 64 * hb
                        nc.tensor.matmul(
                            s_ps[:, hb, :],
                            lhsT=kT_sb[base:base + DH, p, ts],
                            rhs=qT_sb[base:base + DH, p, qs],
                            start=True,
                            stop=True,
                            tile_position=(base, 0),
                        )
                    nc.scalar.activation(
                        out=expair[:, 2 * j:2 * j + 2, :],
                        in_=s_ps,
                        func=mybir.ActivationFunctionType.Exp,
                        scale=SCALE,
                    )

                loop_sched = sched[(p, qc)]
                prevq = []
                for ti in range(0, NTT, 2):
                    expair = exp_pool.tile([P, 4, 512], BF16, name="ex",
                                           tag="ex")
                    qk_exp(ti, expair, 0)
                    qk_exp(ti + 1, expair, 1)
                    # softmax denominators: one DVE fp16 add per pair
                    if ti == 0:
                        nc.vector.tensor_copy(out=acc, in_=expair)
                    else:
                        nc.vector.tensor_add(acc, acc, expair)
                    # previous phase's deferred sums/normalize first, then
                    # PE filler while ScalarE crunches exp
                    if ti == 0 and pending_finish:
                        pending_finish.pop()()
                    for a in loop_sched[ti] + loop_sched[ti + 1]:
                        a()
                    # software-pipelined PV, two pairs behind, so PV's
                    # wait on exp never delays the next scores
                    if len(prevq) == 2:
                        pe, pt = prevq.pop(0)
                        pv_sums(pe, 0, pt)
                        pv_sums(pe, 1, pt + 1)
                    prevq.append((expair, ti))
                for pe, pt in prevq:
                    pv_sums(pe, 0, pt)
                    pv_sums(pe, 1, pt + 1)

                def finish(p=p, qs=qs, acc=acc, pvt=pvt):
                    # reduce fp16 partials over partitions; ones-matmul also
                    # broadcasts each head's sums to its 64 out partitions
                    smt = psp.tile([P, 512], F32, name="smt", tag="pp",
                                   bufs=2)
                    for j in range(2):
                        nc.tensor.matmul(
                            smt[0:DH, :], lhsT=ones64, rhs=acc[:, 2 * j, :],
                            start=(j == 0), stop=(j == 1),
                            tile_position=(0, 0),
                        )
                        nc.tensor.matmul(
                            smt[DH:P, :], lhsT=ones64,
                            rhs=acc[:, 2 * j + 1, :],
                            start=(j == 0), stop=(j == 1),
                            tile_position=(0, 64),
                            skip_group_check=True,
                        )
                    # normalize: attnT = outT * (1/sums)
                    rec = small.tile([P, 512], F32, name="rec", tag="rec")
                    nc.vector.reciprocal_approx_fast(out=rec, in_=smt)
                    nc.vector.tensor_mul(at_sb[:, p, qs], pvt[:, :], rec)
                pending_finish.append(finish)

        # ================= coda: o-projection finals for qc1 ==========
        # flush the last phase's deferred sums/normalize first
        while pending_finish:
            pending_finish.pop()()
        # emit all matmuls first (PE runs ahead), then the copies/stores;
        # pvt's banks are dead here, so alternate pp/pv tags for depth 4
        finals = [oproj_final(m8, 1, ptag=("pv" if m8 % 2 else "pp"),
                              scalar_copy=(m8 % 2 == 0))
                  for m8 in range(D // P)]
        for _, comp in finals:
            for a in comp[:-1]:
                a()
        for _, comp in finals:
            comp[-1]()


_CACHED_NC = None


def build_program():
    global _CACHED_NC
    if _CACHED_NC is not None:
        return _CACHED_NC
    nc = bacc.Bacc(
        "TRN2", target_bir_lowering=False, debug=False, num_devices=NCORES
    )
    # activations / weights are host-packed tile-major: [p, chunk, d, cols]
    xqT = nc.dram_tensor("xqT", [P, NQC, ND, 512], BF16,
                         kind="ExternalInput").ap()
    xkT = nc.dram_tensor("xkT", [P, NTC, ND, 512], BF16,
                         kind="ExternalInput").ap()
    wq0 = nc.dram_tensor("wq0", [P, ND, P], BF16, kind="ExternalInput").ap()
    wq1 = nc.dram_tensor("wq1", [P, ND, F - P], BF16,
                         kind="ExternalInput").ap()
    wk0 = nc.dram_tensor("wk0", [P, ND, P], BF16, kind="ExternalInput").ap()
    wk1 = nc.dram_tensor("wk1", [P, ND, F - P], BF16,
                         kind="ExternalInput").ap()
    wvT = nc.dram_tensor("wvT", [P, ND, F], BF16, kind="ExternalInput").ap()
    woT = nc.dram_tensor("woT", [P, NM, D], BF16, kind="ExternalInput").ap()
    yT = nc.dram_tensor("yT", [D, Q], BF16, kind="ExternalOutput").ap()
    yT2 = nc.dram_tensor("yT2", [D, Q], BF16, kind="ExternalOutput").ap()
    with tile.TileContext(nc) as tc:
        _emit_kernel(nc, tc, xqT, xkT, wq0, wq1, wk0, wk1, wvT, woT, yT, yT2)
    nc.compile()
    _CACHED_NC = nc
    return nc


def _pack_x(xT, nchunks):
    """[D, L] feature-major -> [P, nchunks, ND, 512] tile-major."""
    return np.ascontiguousarray(
        xT.reshape(ND, P, nchunks, 512).transpose(1, 2, 0, 3)
    )


def _pack_w(wT, ncols):
    """[n*P, ncols] -> [P, n, ncols] tile-major."""
    n = wT.shape[0] // P
    return np.ascontiguousarray(wT.reshape(n, P, ncols).transpose(1, 0, 2))


def make_in_maps(q_in, kv_in, Wq, Wk, Wv, Wo):
    """Shard + transpose + cast + tile-pack on host. Core = b*2 + g."""
    in_maps = []
    xqTs, xkTs = [], []
    for b in range(B):
        xqTs.append(_pack_x(q_in[b].T.astype(NPBF16), NQC))
        xkTs.append(_pack_x(kv_in[b].T.astype(NPBF16), NTC))
    w_parts = []
    for g in range(G):
        blk = slice(g * F, (g + 1) * F)
        wq = _pack_w(Wq[blk, :].T.astype(NPBF16), F)
        wk = _pack_w(Wk[blk, :].T.astype(NPBF16), F)
        w_parts.append(
            dict(
                wq0=np.ascontiguousarray(wq[:, :, 0:P]),
                wq1=np.ascontiguousarray(wq[:, :, P:F]),
                wk0=np.ascontiguousarray(wk[:, :, 0:P]),
                wk1=np.ascontiguousarray(wk[:, :, P:F]),
                wvT=_pack_w(Wv[blk, :].T.astype(NPBF16), F),
                woT=_pack_w(Wo[:, blk].T.astype(NPBF16), D),
            )
        )
    for b in range(B):
        for g in range(G):
            m = dict(xqT=xqTs[b], xkT=xkTs[b])
            m.update(w_parts[g])
            in_maps.append(m)
    return in_maps


def assemble_output(results):
    """results: per-core dicts with bf16 'yT' (pair-3 final) and 'yT2'
    (pairs 0-2 partial), both [D, Q]; host sums and transposes."""
    out = np.empty((B, Q, D), dtype=np.float32)
    for b in range(B):
        acc = (
            results[2 * b]["yT"].astype(np.float32)
            + results[2 * b]["yT2"].astype(np.float32)
            + results[2 * b + 1]["yT"].astype(np.float32)
            + results[2 * b + 1]["yT2"].astype(np.float32)
        )
        out[b] = acc.T
    return out


def kernel(q_in, kv_in, Wq, Wk, Wv, Wo):
    q_in = np.asarray(q_in, dtype=np.float32)
    kv_in = np.asarray(kv_in, dtype=np.float32)
    Wq = np.asarray(Wq, dtype=np.float32)
    Wk = np.asarray(Wk, dtype=np.float32)
    Wv = np.asarray(Wv, dtype=np.float32)
    Wo = np.asarray(Wo, dtype=np.float32)
    nc = build_program()
    in_maps = make_in_maps(q_in, kv_in, Wq, Wk, Wv, Wo)
    res = run_bass_kernel_spmd(nc, in_maps, list(range(NCORES)))
    return assemble_output(res.results)


# revision 21
# speedup vs baseline: 1.0404x; 1.0404x over previous
"""Trainium2 Bass kernel for nn_CrossAttention (B=4, Q=1024, T=4096, D=1024, H=16).

Sharding: core = b*2 + g  (b in 0..3 batches, g in 0..1 head-groups of 8 heads).
Each core computes, for its (batch, head-group):
  qT = (Wq_g @ x_q.T)          [512, Q]   (feature-major; head pairs stacked)
  kT = (Wk_g @ x_kv.T)         [512, T]
  v  = (x_kv @ Wv_g.T)         [T, 512]
  sT = k_h @ q_h.T             [T, Q] per head  (scores transposed)
  p  = exp(sT / 8)             (softmax w/o max-subtraction; scores ~N(0,1))
  outT_h = v_h.T @ p           (PV accumulated in PSUM)
  sums_h = column-sums of p    (DVE+GpSimd fp16 running accumulator + one PE
                                ones-matmul reduce, which also broadcasts)
  attnT_h = outT_h * (1/sums_h)
  yT += Wo[:, gblock].T.T @ attnT  (pairs 0-2 partial and pair-3 final stored
                                    separately in bf16; host sums)
Host sums the partials per batch and transposes.

The schedule is exp-paced: ScalarE's exp throughput (~1.1us per t-tile of two
heads) is the hard floor, so projection work is spread so that no phase's PE
backlog exceeds its exp budget (~35.6us).  v-proj is sliced by head-pair
columns (pair 0 inside phase (0,0), pair 1 in phases 2-3, pairs 2-3 in
phases 4-5); k-proj chunks carry per-chunk deadlines spanning two phases per
pair; o-projection partials are DMA'd out (bf16) as soon as their inputs
exist and summed on the host.  A chain of dummy matmuls at the top warms the
PE HAM clock gate during the initial DMA wait.
"""

import os
import sys

import numpy as np

# The device can enter a persistent ~20% clock-throttle state after long
# sessions; a core reset at runtime init restores nominal speed.
os.environ.setdefault("NEURON_RT_RESET_CORES", "1")

for _p in ("/opt/trn_rl_repo",):
    if _p not in sys.path:
        sys.path.insert(0, _p)

import ml_dtypes

import concourse.bass as bass
import concourse.tile as tile
from concourse import bacc, mybir
from concourse.bass_utils import run_bass_kernel_spmd

BF16 = mybir.dt.bfloat16
F16 = mybir.dt.float16
F32 = mybir.dt.float32
NPBF16 = np.dtype(ml_dtypes.bfloat16)

D = 1024          # model dim
Q = 1024          # query length
T = 4096          # kv length
B = 4             # batch
H = 16            # heads
DH = 64           # head dim
NCORES = 8
G = 2             # head groups (cores per batch)
F = D // G        # features per core = 512
P = 128
ND = D // P       # 8 d-tiles (contraction tiles for projections)
NM = F // P       # 4 feature tiles (head pairs)
NQC = Q // 512    # 2 query chunks
NTC = T // 512    # 8 kv chunks
NTT = T // P      # 32 kv tiles
SCALE = DH ** -0.5


def _emit_kernel(nc, tc, xqT, xkT, wq0, wq1, wk0, wk1, wv0, wv1, woT, yT, yT2):
    from contextlib import ExitStack

    ctx = ExitStack()
    with ctx:
        wp = ctx.enter_context(tc.tile_pool(name="wp", bufs=1))
        xp = ctx.enter_context(tc.tile_pool(name="xp", bufs=4))
        xqp = ctx.enter_context(tc.tile_pool(name="xqp", bufs=2))
        st = ctx.enter_context(tc.tile_pool(name="st", bufs=1))
        exp_pool = ctx.enter_context(tc.tile_pool(name="exp", bufs=4))
        accp = ctx.enter_context(tc.tile_pool(name="accp", bufs=2))
        small = ctx.enter_context(tc.tile_pool(name="small", bufs=2))
        yop = ctx.enter_context(tc.tile_pool(name="yop", bufs=4))
        psp = ctx.enter_context(tc.tile_pool(name="psp", bufs=1, space="PSUM"))

        # ---- resident weights / activations ----
        wq0_sb = wp.tile([P, ND, P], BF16, name="wq0_sb", tag="wq0")
        wq1_sb = wp.tile([P, ND, F - P], BF16, name="wq1_sb", tag="wq1")
        wk0_sb = wp.tile([P, ND, P], BF16, name="wk0_sb", tag="wk0")
        wk1_sb = wp.tile([P, ND, F - P], BF16, name="wk1_sb", tag="wk1")
        wv0_sb = wp.tile([P, ND, P], BF16, name="wv0_sb", tag="wv0")
        wv1_sb = wp.tile([P, ND, F - P], BF16, name="wv1_sb", tag="wv1")
        wo_sb = wp.tile([P, NM, D], BF16, name="wo_sb", tag="wo")
        qT_sb = st.tile([P, NM, Q], BF16, name="qT_sb", tag="qT")
        kT_sb = st.tile([P, NM, T], BF16, name="kT_sb", tag="kT")
        v_sb = st.tile([P, NTT, F], BF16, name="v_sb", tag="v")
        at_sb = st.tile([P, NM, Q], BF16, name="at_sb", tag="at")
        ones64 = st.tile([P, DH], F16, name="ones64", tag="ones")

        def wsel_q(m):
            return (wq0_sb, 0) if m == 0 else (wq1_sb, (m - 1) * P)

        def wsel_k(p):
            return (wk0_sb, 0) if p == 0 else (wk1_sb, (p - 1) * P)

        def xk_dma(tc_i):
            """Load one 512-col chunk of xkT; returns the tile."""
            xk2 = xp.tile([P, ND, 512], BF16, name="xk2", tag="xk2")
            nc.sync.dma_start(out=xk2, in_=xkT[:, tc_i:tc_i + 1, :, :])
            return xk2

        # ---- projection emitters: (pre_action, [compute actions]) ----
        # xk_get: callable resolving the input chunk tile at run time (for
        # chunk sharing between k-proj and v-proj); if None, a private DMA
        # pre-action is returned.
        def kproj_chunk(p, tc_i, xk_get=None, c0=0, c1=512):
            state = {}

            def dma():
                state["xk"] = xk_dma(tc_i)
            getx = xk_get if xk_get is not None else (lambda: state["xk"])

            comp = []

            def alloc():
                state["pk"] = psp.tile([P, c1 - c0], F32, name="pk", tag="pp",
                                       bufs=2)

            comp.append(alloc)
            w_sb, woff = wsel_k(p)
            for d in range(ND):
                def mm(d=d):
                    nc.tensor.matmul(
                        state["pk"],
                        lhsT=w_sb[:, d, woff:woff + P],
                        rhs=getx()[:, d, c0:c1],
                        start=(d == 0),
                        stop=(d == ND - 1),
                    )
                comp.append(mm)

            def cp():
                nc.vector.tensor_copy(
                    out=kT_sb[:, p, tc_i * 512 + c0:tc_i * 512 + c1],
                    in_=state["pk"],
                )
            comp.append(cp)
            return (None if xk_get is not None else dma), comp

        def vproj_cols(tc_i, c0, c1, xk_get=None):
            """v columns [c0:c1) for one 512-row chunk.  For width<=128 all
            4 t-tiles accumulate in one [P, 4, w] psum bank with a single
            strided copy out; for wider slices, per-t-tile psums."""
            w = c1 - c0
            state = {}

            def dma():
                state["xk"] = xk_dma(tc_i)
            getx = xk_get if xk_get is not None else (lambda: state["xk"])

            comp = []
            if w <= P:
                def alloc():
                    state["ps"] = psp.tile([P, 4, w], F32, name="pv0",
                                           tag="pp", bufs=2)
                comp.append(alloc)
                for j in range(4):
                    for d in range(ND):
                        def mm(j=j, d=d):
                            nc.tensor.matmul(
                                state["ps"][:, j, :],
                                lhsT=getx()[:, d, j * P:(j + 1) * P],
                                rhs=(wv0_sb[:, d, :] if c1 <= P else
                                     wv1_sb[:, d, c0 - P:c1 - P]),
                                start=(d == 0),
                                stop=(d == ND - 1),
                            )
                        comp.append(mm)

                def cp():
                    nc.vector.tensor_copy(
                        out=v_sb[:, tc_i * 4:(tc_i + 1) * 4, c0:c1],
                        in_=state["ps"],
                    )
                comp.append(cp)
            else:
                for j in range(4):
                    def alloc(j=j):
                        state[j] = psp.tile([P, w], F32, name="pvr",
                                            tag="pp", bufs=2)
                    comp.append(alloc)
                    for d in range(ND):
                        def mm(j=j, d=d):
                            nc.tensor.matmul(
                                state[j],
                                lhsT=getx()[:, d, j * P:(j + 1) * P],
                                rhs=(wv0_sb[:, d, :] if c1 <= P else
                                     wv1_sb[:, d, c0 - P:c1 - P]),
                                start=(d == 0),
                                stop=(d == ND - 1),
                            )
                        comp.append(mm)

                    def cp(j=j):
                        nc.vector.tensor_copy(
                            out=v_sb[:, tc_i * 4 + j, c0:c1], in_=state[j]
                        )
                    comp.append(cp)
            return (None if xk_get is not None else dma), comp

        def qproj_dma(qc):
            xq_t = xqp.tile([P, ND, 512], BF16, name="xq_t", tag="xq")
            nc.sync.dma_start(out=xq_t, in_=xqT[:, qc:qc + 1, :, :])
            return xq_t

        def qproj_m(qc, m, xq_get):
            """Compute actions for one head-pair column block of q-proj."""
            state = {}
            comp = []

            def alloc():
                state["pq"] = psp.tile([P, 512], F32, name="pq", tag="pp",
                                       bufs=2)
            comp.append(alloc)
            w_sb, woff = wsel_q(m)
            for d in range(ND):
                def mm(d=d):
                    nc.tensor.matmul(
                        state["pq"],
                        lhsT=w_sb[:, d, woff:woff + P],
                        rhs=xq_get()[:, d, :],
                        start=(d == 0),
                        stop=(d == ND - 1),
                    )
                comp.append(mm)

            def cp():
                nc.vector.tensor_copy(
                    out=qT_sb[:, m, qc * 512:(qc + 1) * 512],
                    in_=state["pq"],
                )
            comp.append(cp)
            return comp

        # o-projection: partial = pairs 0-2 -> bf16 -> yT2 (host adds);
        # final = pair 3 -> bf16 -> yT.  No cross-dependencies, so partials
        # stream out as soon as their at_sb pairs exist.
        def oproj_partial(m8, qc):
            state = {}
            comp = []

            def alloc():
                state["py"] = psp.tile([P, 512], F32, name="pyp", tag="pp",
                                       bufs=2)
            comp.append(alloc)
            for k in range(NM - 1):
                def mm(k=k):
                    nc.tensor.matmul(
                        state["py"],
                        lhsT=wo_sb[:, k, m8 * P:(m8 + 1) * P],
                        rhs=at_sb[:, k, qc * 512:(qc + 1) * 512],
                        start=(k == 0),
                        stop=(k == NM - 2),
                    )
                comp.append(mm)

            def cp():
                yp = yop.tile([P, 512], BF16, name="yp", tag="y")
                nc.vector.tensor_copy(out=yp, in_=state["py"])
                nc.sync.dma_start(
                    out=yT2[m8 * P:(m8 + 1) * P, qc * 512:(qc + 1) * 512],
                    in_=yp,
                )
            comp.append(cp)
            return None, comp

        def oproj_final(m8, qc, ptag="pp", scalar_copy=False):
            state = {}
            comp = []

            def alloc():
                state["py"] = psp.tile([P, 512], F32, name="pyf", tag=ptag,
                                       bufs=2)
            comp.append(alloc)

            def mm():
                nc.tensor.matmul(
                    state["py"],
                    lhsT=wo_sb[:, NM - 1, m8 * P:(m8 + 1) * P],
                    rhs=at_sb[:, NM - 1, qc * 512:(qc + 1) * 512],
                    start=True,
                    stop=True,
                )
            comp.append(mm)

            def st_dma():
                y_t = yop.tile([P, 512], BF16, name="y_t", tag="y")
                if scalar_copy:
                    # coda: idle ScalarE moves PSUM->SBUF
                    nc.scalar.copy(out=y_t, in_=state["py"])
                else:
                    nc.vector.tensor_copy(out=y_t, in_=state["py"])
                nc.sync.dma_start(
                    out=yT[m8 * P:(m8 + 1) * P, qc * 512:(qc + 1) * 512],
                    in_=y_t,
                )
            comp.append(st_dma)
            return None, comp

        def run(pre, comp):
            if pre is not None:
                pre()
            for a in comp:
                a()

        def spread(pairs, nsteps, lead=4, s0=0):
            """Evenly distribute (pre, comp) groups over steps [s0, s0+nsteps);
            pre (DMA) actions are placed `lead` slots before the group's
            first compute action."""
            sched = [[] for _ in range(NTT)]
            total = sum(len(c) for _, c in pairs) or 1
            pos = 0
            for pre, comp in pairs:
                first = s0 + (pos * nsteps) // total
                if pre is not None:
                    sched[max(0, min(NTT - 1, first - lead))].append(pre)
                for a in comp:
                    sched[min(NTT - 1, s0 + (pos * nsteps) // total)
                          ].append(a)
                    pos += 1
            return sched

        def merge(*scheds):
            out = [[] for _ in range(NTT)]
            for s in scheds:
                for i, acts in enumerate(s):
                    out[min(i, NTT - 1)].extend(acts)
            return out

        # ================= prologue =================
        # HAM warmup: dummy matmuls on ones64 keep the PE busy through the
        # initial DMA wait so the first real matmuls run at 2.4 GHz.
        nc.vector.memset(ones64, 1.0)
        wps = psp.tile([P, 512], F32, name="warm", tag="pp", bufs=2)
        for i in range(96):
            nc.tensor.matmul(
                wps[0:DH, 0:DH], lhsT=ones64[:, 0:DH], rhs=ones64[:, 0:DH],
                start=True, stop=True,
            )
        # critical-path DMAs first (contiguous tensors, cheap descriptors)
        # all input loads on the Sync HWDGE queue (it alone sustains
        # ~365 GB/s; the ScalarE/GpSimd queues are far slower), issued in
        # deadline order: the critical first-exp chain first.
        xq_hold = {0: None, 1: None}
        xq_hold[0] = qproj_dma(0)
        nc.sync.dma_start(out=wq0_sb, in_=wq0[:, :, :])
        xk_hold = {0: xk_dma(0)}
        nc.sync.dma_start(out=wk0_sb, in_=wk0[:, :, :])
        nc.sync.dma_start(out=wv0_sb, in_=wv0[:, :, :])
        xk_hold[1] = xk_dma(1)
        nc.sync.dma_start(out=wk1_sb, in_=wk1[:, :, :])
        nc.sync.dma_start(out=wv1_sb, in_=wv1[:, :, :])
        nc.sync.dma_start(out=wq1_sb, in_=wq1[:, :, :])
        # q-proj pair 0 (xq lands first), then k-proj chunk 0 in two pieces
        getx0 = lambda: xk_hold[0]
        for a in qproj_m(0, 0, lambda: xq_hold[0]):
            a()
        run(*kproj_chunk(0, 0, xk_get=getx0, c0=0, c1=128))
        run(*kproj_chunk(0, 0, xk_get=getx0, c0=128, c1=512))

        # ================= filler schedules ==================
        def xk_load(c, hold):
            def act(c=c):
                hold[c] = xk_dma(c)
            return act

        def holder(hold, c):
            return lambda: hold[c]

        # phase (0,0): deadline-driven: k-proj(p0) chunk c by step 4c,
        # v-proj pair0 chunk c by step 4c+2; one shared xk load per chunk.
        p0 = [[] for _ in range(NTT)]

        def qp1dma():
            xq_hold[1] = qproj_dma(1)
        p0[0].append(qp1dma)
        vp0 = [vproj_cols(c, 0, P, xk_get=holder(xk_hold, c))
               for c in range(NTC)]
        # chunk 0 lands in steps 0-1 (PV consumes v tiles 0-3 from step 4)
        n0 = len(vp0[0][1])
        p0[0].extend(vp0[0][1][:(n0 + 1) // 2])
        p0[1].extend(vp0[0][1][(n0 + 1) // 2:])
        for c in range(1, NTC):
            if c >= 2:
                p0[max(0, 4 * c - 8)].append(xk_load(c, xk_hold))
            _, comp = kproj_chunk(0, c, xk_get=holder(xk_hold, c))
            base = 4 * (c - 1)
            n = len(comp)
            for si in range(4):
                lo, hi = (n * si) // 4, (n * (si + 1)) // 4
                p0[base + si].extend(comp[lo:hi])
            # v pair0 chunk c: steps 4c-2 .. 4c+1
            _, comp_v = vp0[c]
            nv = len(comp_v)
            for si in range(4):
                lo, hi = (nv * si) // 4, (nv * (si + 1)) // 4
                p0[min(NTT - 1, 4 * c - 2 + si)].extend(comp_v[lo:hi])
        # deferred q-proj for qc1 pair 0 (due end of this phase)
        for i, a in enumerate(qproj_m(1, 0, lambda: xq_hold[1])):
            p0[26 + (i * 6) // 10].append(a)
        sched = {(0, 0): p0}

        # phase (0,1): combined [kp1[c] + vp1[c]] groups for chunks 0-3
        # sharing one xk load each, + q-proj m1 qc0 + wo DMA.
        xk_h2 = {}

        def kv_group(p, c, vc0, vc1, hold):
            pre = xk_load(c, hold)
            _, kcomp = kproj_chunk(p, c, xk_get=holder(hold, c))
            _, vcomp = vproj_cols(c, vc0, vc1, xk_get=holder(hold, c))
            return pre, kcomp + vcomp

        sched[(0, 1)] = merge(
            spread([kv_group(1, c, P, 2 * P, xk_h2) for c in range(4)],
                   28, s0=0),
            spread([(None, qproj_m(0, 1, lambda: xq_hold[0])),
                    (lambda: nc.sync.dma_start(out=wo_sb, in_=woT[:, :, :]),
                     [])], 4, s0=28),
        )
        # phase (1,0): kp1+vp1 chunks 4-7 (kp1 deadline 4c), kp2 chunks 0-1,
        # q m1 qc1 (due before phase (1,1))
        sched[(1, 0)] = merge(
            spread([kv_group(1, c, P, 2 * P, xk_h2) for c in range(4, NTC)],
                   24, s0=0),
            spread([(None, qproj_m(1, 1, lambda: xq_hold[1]))], 4, s0=26),
        )
        # phases (1,1)/(2,0): kp2 chunk c and vp23 chunk c share one xk
        # load; chunks 0-3 in (1,1), 4-7 in (2,0) (deadlines 4c there)
        xk_h3 = {}
        sched[(1, 1)] = merge(
            spread([kv_group(2, c, 2 * P, F, xk_h3) for c in range(4)],
                   28, s0=0),
            spread([(None, qproj_m(0, 2, lambda: xq_hold[0]))], 4, s0=28),
        )
        sched[(2, 0)] = merge(
            spread([kv_group(2, c, 2 * P, F, xk_h3) for c in range(4, NTC)],
                   26, s0=0),
            spread([(None, qproj_m(1, 2, lambda: xq_hold[1]))], 4, s0=28),
        )
        # phase (2,1): kp3 chunks 0-5, q m3 qc0, o-partial qc0
        sched[(2, 1)] = merge(
            spread([kproj_chunk(3, c) for c in range(0, 6)], 18, s0=0),
            spread([(None, qproj_m(0, 3, lambda: xq_hold[0]))], 4, s0=18),
            spread([oproj_partial(m8, 0) for m8 in range(D // P)], 10,
                   s0=22),
        )
        # phase (3,0): kp3 chunks 6-7 (deadlines 24/28), q m3 qc1,
        # o-partial qc1
        sched[(3, 0)] = merge(
            spread([kproj_chunk(3, c) for c in range(6, NTC)], 8, s0=14),
            spread([oproj_partial(m8, 1) for m8 in range(D // P)], 14, s0=0),
            spread([(None, qproj_m(1, 3, lambda: xq_hold[1]))], 6, s0=24),
        )
        # phase (3,1): o-final qc0
        sched[(3, 1)] = spread(
            [oproj_final(m8, 0) for m8 in range(D // P)], NTT)

        # ================= attention (software-pipelined) ========
        # The previous phase's sums-reduce + normalize is deferred into the
        # current phase (emitted right after the first two QK groups) so
        # the next phase's first exp is never gated behind it.
        pending_finish = []
        for p in range(NM):
            for qc in range(NQC):
                qs = slice(qc * 512, (qc + 1) * 512)
                pvt = psp.tile([P, 512], F32, name="pvt", tag="pv", bufs=2)
                acc = accp.tile([P, 4, 512], F16, name="acc", tag="acc")

                def pv_sums(ex, j, t, p=p, pvt=pvt):
                    nc.tensor.matmul(
                        pvt[0:DH, :],
                        lhsT=v_sb[:, t, p * P:p * P + DH],
                        rhs=ex[:, 2 * j, :],
                        start=(t == 0),
                        stop=(t == NTT - 1),
                        tile_position=(0, 0),
                    )
                    nc.tensor.matmul(
                        pvt[DH:P, :],
                        lhsT=v_sb[:, t, p * P + DH:(p + 1) * P],
                        rhs=ex[:, 2 * j + 1, :],
                        start=(t == 0),
                        stop=(t == NTT - 1),
                        tile_position=(0, 64),
                        skip_group_check=True,
                    )

                def qk_exp(t, expair, j):
                    ts = slice(t * P, (t + 1) * P)
                    s_ps = psp.tile([P, 2, 512], F32, name="s_ps", tag="ps",
                                    bufs=2)
                    for hb in range(2):
                        base = 64 * hb
                        nc.tensor.matmul(
                            s_ps[:, hb, :],
                            lhsT=kT_sb[base:base + DH, p, ts],
                            rhs=qT_sb[base:base + DH, p, qs],
                            start=True,
                            stop=True,
                            tile_position=(base, 0),
                        )
                    nc.scalar.activation(
                        out=expair[:, 2 * j:2 * j + 2, :],
                        in_=s_ps,
                        func=mybir.ActivationFunctionType.Exp,
                        scale=SCALE,
                    )

                loop_sched = sched[(p, qc)]
                prevq = []
                for ti in range(0, NTT, 2):
                    expair = exp_pool.tile([P, 4, 512], BF16, name="ex",
                                           tag="ex")
                    qk_exp(ti, expair, 0)
                    qk_exp(ti + 1, expair, 1)
                    # softmax denominators: one DVE fp16 add per pair
                    if ti == 0:
                        nc.vector.tensor_copy(out=acc, in_=expair)
                    else:
                        nc.vector.tensor_add(acc, acc, expair)
                    # previous phase's deferred sums/normalize first, then
                    # PE filler while ScalarE crunches exp
                    if ti == 0 and pending_finish:
                        pending_finish.pop()()
                    for a in loop_sched[ti] + loop_sched[ti + 1]:
                        a()
                    # software-pipelined PV, two pairs behind, so PV's
                    # wait on exp never delays the next scores
                    if len(prevq) == 2:
                        pe, pt = prevq.pop(0)
                        pv_sums(pe, 0, pt)
                        pv_sums(pe, 1, pt + 1)
                    prevq.append((expair, ti))
                def finish(p=p, qs=qs, acc=acc, pvt=pvt, prevq=prevq,
                           pv_sums=pv_sums):
                    # drain the software-pipelined PV tail first
                    for pe, pt in prevq:
                        pv_sums(pe, 0, pt)
                        pv_sums(pe, 1, pt + 1)
                    # reduce fp16 partials over partitions; ones-matmul also
                    # broadcasts each head's sums to its 64 out partitions
                    smt = psp.tile([P, 512], F32, name="smt", tag="pp",
                                   bufs=2)
                    for j in range(2):
                        nc.tensor.matmul(
                            smt[0:DH, :], lhsT=ones64, rhs=acc[:, 2 * j, :],
                            start=(j == 0), stop=(j == 1),
                            tile_position=(0, 0),
                        )
                        nc.tensor.matmul(
                            smt[DH:P, :], lhsT=ones64,
                            rhs=acc[:, 2 * j + 1, :],
                            start=(j == 0), stop=(j == 1),
                            tile_position=(0, 64),
                            skip_group_check=True,
                        )
                    # normalize: attnT = outT * (1/sums)
                    rec = small.tile([P, 512], F32, name="rec", tag="rec")
                    nc.vector.reciprocal_approx_fast(out=rec, in_=smt)
                    nc.vector.tensor_mul(at_sb[:, p, qs], pvt[:, :], rec)
                pending_finish.append(finish)

        # ================= coda: o-projection finals for qc1 ==========
        # flush the last phase's deferred sums/normalize first
        while pending_finish:
            pending_finish.pop()()
        # emit all matmuls first (PE runs ahead), then the copies/stores;
        # pvt's banks are dead here, so alternate pp/pv tags for depth 4
        finals = [oproj_final(m8, 1, ptag=("pv" if m8 % 2 else "pp"),
                              scalar_copy=(m8 % 2 == 0))
                  for m8 in range(D // P)]
        for _, comp in finals:
            for a in comp[:-1]:
                a()
        for _, comp in finals:
            comp[-1]()


_CACHED_NC = None


def build_program():
    global _CACHED_NC
    if _CACHED_NC is not None:
        return _CACHED_NC
    nc = bacc.Bacc(
        "TRN2", target_bir_lowering=False, debug=False, num_devices=NCORES
    )
    # activations / weights are host-packed tile-major: [p, chunk, d, cols]
    xqT = nc.dram_tensor("xqT", [P, NQC, ND, 512], BF16,
                         kind="ExternalInput").ap()
    xkT = nc.dram_tensor("xkT", [P, NTC, ND, 512], BF16,
                         kind="ExternalInput").ap()
    wq0 = nc.dram_tensor("wq0", [P, ND, P], BF16, kind="ExternalInput").ap()
    wq1 = nc.dram_tensor("wq1", [P, ND, F - P], BF16,
                         kind="ExternalInput").ap()
    wk0 = nc.dram_tensor("wk0", [P, ND, P], BF16, kind="ExternalInput").ap()
    wk1 = nc.dram_tensor("wk1", [P, ND, F - P], BF16,
                         kind="ExternalInput").ap()
    wv0 = nc.dram_tensor("wv0", [P, ND, P], BF16, kind="ExternalInput").ap()
    wv1 = nc.dram_tensor("wv1", [P, ND, F - P], BF16,
                         kind="ExternalInput").ap()
    woT = nc.dram_tensor("woT", [P, NM, D], BF16, kind="ExternalInput").ap()
    yT = nc.dram_tensor("yT", [D, Q], BF16, kind="ExternalOutput").ap()
    yT2 = nc.dram_tensor("yT2", [D, Q], BF16, kind="ExternalOutput").ap()
    with tile.TileContext(nc) as tc:
        _emit_kernel(nc, tc, xqT, xkT, wq0, wq1, wk0, wk1, wv0, wv1, woT, yT, yT2)
    nc.compile()
    _CACHED_NC = nc
    return nc


def _pack_x(xT, nchunks):
    """[D, L] feature-major -> [P, nchunks, ND, 512] tile-major."""
    return np.ascontiguousarray(
        xT.reshape(ND, P, nchunks, 512).transpose(1, 2, 0, 3)
    )


def _pack_w(wT, ncols):
    """[n*P, ncols] -> [P, n, ncols] tile-major."""
    n = wT.shape[0] // P
    return np.ascontiguousarray(wT.reshape(n, P, ncols).transpose(1, 0, 2))


def make_in_maps(q_in, kv_in, Wq, Wk, Wv, Wo):
    """Shard + transpose + cast + tile-pack on host. Core = b*2 + g."""
    in_maps = []
    xqTs, xkTs = [], []
    for b in range(B):
        xqTs.append(_pack_x(q_in[b].T.astype(NPBF16), NQC))
        xkTs.append(_pack_x(kv_in[b].T.astype(NPBF16), NTC))
    w_parts = []
    for g in range(G):
        blk = slice(g * F, (g + 1) * F)
        wq = _pack_w(Wq[blk, :].T.astype(NPBF16), F)
        wk = _pack_w(Wk[blk, :].T.astype(NPBF16), F)
        w_parts.append(
            dict(
                wq0=np.ascontiguousarray(wq[:, :, 0:P]),
                wq1=np.ascontiguousarray(wq[:, :, P:F]),
                wk0=np.ascontiguousarray(wk[:, :, 0:P]),
                wk1=np.ascontiguousarray(wk[:, :, P:F]),
                wv0=np.ascontiguousarray(
                    _pack_w(Wv[blk, :].T.astype(NPBF16), F)[:, :, 0:P]),
                wv1=np.ascontiguousarray(
                    _pack_w(Wv[blk, :].T.astype(NPBF16), F)[:, :, P:F]),
                woT=_pack_w(Wo[:, blk].T.astype(NPBF16), D),
            )
        )
    for b in range(B):
        for g in range(G):
            m = dict(xqT=xqTs[b], xkT=xkTs[b])
            m.update(w_parts[g])
            in_maps.append(m)
    return in_maps


def assemble_output(results):
    """results: per-core dicts with bf16 'yT' (pair-3 final) and 'yT2'
    (pairs 0-2 partial), both [D, Q]; host sums and transposes."""
    out = np.empty((B, Q, D), dtype=np.float32)
    for b in range(B):
        acc = (
            results[2 * b]["yT"].astype(np.float32)
            + results[2 * b]["yT2"].astype(np.float32)
            + results[2 * b + 1]["yT"].astype(np.float32)
            + results[2 * b + 1]["yT2"].astype(np.float32)
        )
        out[b] = acc.T
    return out


def kernel(q_in, kv_in, Wq, Wk, Wv, Wo):
    q_in = np.asarray(q_in, dtype=np.float32)
    kv_in = np.asarray(kv_in, dtype=np.float32)
    Wq = np.asarray(Wq, dtype=np.float32)
    Wk = np.asarray(Wk, dtype=np.float32)
    Wv = np.asarray(Wv, dtype=np.float32)
    Wo = np.asarray(Wo, dtype=np.float32)
    nc = build_program()
    in_maps = make_in_maps(q_in, kv_in, Wq, Wk, Wv, Wo)
    res = run_bass_kernel_spmd(nc, in_maps, list(range(NCORES)))
    return assemble_output(res.results)
